# revision 11
# baseline (speedup 1.0000x reference)
"""MultiHeadAttention TRN2 kernel — 8-way head-parallel (2 heads/core).

Sharding: core c owns heads 2c,2c+1 == columns c*128:(c+1)*128 of Wq/Wk/Wv
and rows c*128:(c+1)*128 of Wo. Each core computes its heads' projections,
attention, and a partial output projection; the host sums the 8 partials.

Device math (per core), all matmuls bf16 with f32 PSUM accumulation:
  qT/kT = (Wq_c^T x^T + bq_c)          feature-major [128, ROWS]
  v     = x^T^T Wv_c                   position-major [ROWS, 128]  (bv dropped:
                                        softmax rows sum to 1 => bv enters as the
                                        constant row bv@Wo, added on host)
  scoresT[kv, q] = kT^T qT             per (b, h), exp via ACT with scale=1/8
  attn_out[d, q] = sum_kv v_aug[kv, d] expT[kv, q]   (v_aug has a ones column
                                        => row 64 accumulates the softmax denom)
  attn = attn_out / denom              (DVE recip + PE broadcast + DVE mul)
  out_partial[q, e] = attn_cat^T Wo_c  f32 out
"""

import sys

sys.path.insert(0, "/opt/trn_rl_repo")

import numpy as np
import ml_dtypes

import concourse.bass as bass
from concourse import bacc
import concourse.mybir as mybir
from concourse.tile import TileContext
from concourse.bass_utils import run_bass_kernel_spmd

BF16 = mybir.dt.bfloat16
F32 = mybir.dt.float32
F32R = mybir.dt.float32r
AF = mybir.ActivationFunctionType

EMBED = 1024
HEADS = 16
HEAD_DIM = 64
N_CORES = 8
DC = 128  # feature columns per core (2 heads * 64)
NEC = 8  # contraction chunks of 128 over EMBED


def build_nc(B=4, S=2048):
    ROWS = B * S
    NRC = ROWS // 512  # row chunks for projections
    NQC = S // 512  # q chunks per (b, h)
    NKV = S // 128  # kv tiles per batch
    nc = bacc.Bacc("TRN2", target_bir_lowering=False)

    qT_d = nc.declare_dram_parameter("qT", [EMBED, ROWS], BF16, isOutput=False)
    kT_d = nc.declare_dram_parameter("kT", [EMBED, ROWS], BF16, isOutput=False)
    vT_d = nc.declare_dram_parameter("vT", [EMBED, ROWS], BF16, isOutput=False)
    wq_d = nc.declare_dram_parameter("wq", [NEC, 128, DC], BF16, isOutput=False)
    wk_d = nc.declare_dram_parameter("wk", [NEC, 128, DC], BF16, isOutput=False)
    wv_d = nc.declare_dram_parameter("wv", [NEC, 128, DC], BF16, isOutput=False)
    bq_d = nc.declare_dram_parameter("bq", [DC, 1], F32, isOutput=False)
    bk_d = nc.declare_dram_parameter("bk", [DC, 1], F32, isOutput=False)
    wo_d = nc.declare_dram_parameter("wo", [DC, EMBED], BF16, isOutput=False)
    out_d = nc.declare_dram_parameter("out", [ROWS, EMBED], F32, isOutput=True)

    with TileContext(nc) as tc:
        with (
            tc.tile_pool(name="const", bufs=1) as cpool,
            tc.tile_pool(name="big", bufs=2) as big,
            tc.tile_pool(name="xin", bufs=16) as xin,
            tc.tile_pool(name="expp", bufs=12) as expp,
            tc.tile_pool(name="ev", bufs=4) as evp,
            tc.tile_pool(name="ot", bufs=3) as otp,
            tc.tile_pool(name="ps", bufs=1, space="PSUM") as ps,
        ):
            # --- weights / constants ---
            wq_sb = cpool.tile([128, NEC * DC], BF16, tag="wq")
            wk_sb = cpool.tile([128, NEC * DC], BF16, tag="wk")
            wv_sb = cpool.tile([128, NEC * DC], BF16, tag="wv")
            for ec in range(NEC):
                nc.sync.dma_start(out=wq_sb[:, ec * DC : (ec + 1) * DC], in_=wq_d[ec])
                nc.sync.dma_start(out=wk_sb[:, ec * DC : (ec + 1) * DC], in_=wk_d[ec])
                nc.sync.dma_start(out=wv_sb[:, ec * DC : (ec + 1) * DC], in_=wv_d[ec])
            wo_sb = cpool.tile([128, EMBED], BF16, tag="wo")
            nc.sync.dma_start(out=wo_sb[:], in_=wo_d[:])
            bq_sb = cpool.tile([128, 1], F32, tag="bq")
            bk_sb = cpool.tile([128, 1], F32, tag="bk")
            nc.sync.dma_start(out=bq_sb[:], in_=bq_d[:])
            nc.sync.dma_start(out=bk_sb[:], in_=bk_d[:])
            ones_sb = cpool.tile([1, 64], F32, tag="ones")
            nc.vector.memset(ones_sb[:], 1.0)

            NKV2 = NKV // 2  # paired kv tiles (exp processes two at once)

            qT_sb, kT_sb, v_sb, attn_sb = {}, {}, {}, {}

            def alloc_batch(b):
                qT_sb[b] = big.tile([128, S], BF16, tag="qTs", name=f"qTs{b}")
                kT_sb[b] = big.tile([128, S], BF16, tag="kTs", name=f"kTs{b}")
                v_sb[b] = big.tile([128, NKV * 2 * 65], BF16, tag="vs", name=f"vs{b}")
                attn_sb[b] = big.tile([128, S], BF16, tag="attns", name=f"attns{b}")
                nc.vector.memset(v_sb[b][:], 1.0)  # ones col (idx 64) per 65-block

            def emit_qk_proj(b, which):
                src_d, wsb, bsb = (
                    (qT_d, wq_sb, bq_sb) if which == "q" else (kT_d, wk_sb, bk_sb)
                )
                dst = (qT_sb if which == "q" else kT_sb)[b]
                xt = []
                for ec in range(NEC):
                    for hf in range(2):
                        t = xin.tile([128, S // 2], BF16, tag="xin", name=f"x{which}{b}e{ec}h{hf}")
                        nc.sync.dma_start(
                            out=t[:],
                            in_=src_d[
                                ec * 128 : (ec + 1) * 128,
                                b * S + hf * (S // 2) : b * S + (hf + 1) * (S // 2),
                            ],
                        )
                        xt.append((ec, hf, t))
                tiles = {(ec, hf): t for ec, hf, t in xt}
                rcph = max(1, S // 2 // 512)
                for rc in range(S // 512):
                    hf, off = rc // rcph, (rc % rcph) * 512
                    pt = ps.tile([128, 512], F32, tag="misc", bufs=2, name="pt")
                    for ec in range(NEC):
                        nc.tensor.matmul(
                            pt[:],
                            wsb[:, ec * DC : (ec + 1) * DC],
                            tiles[(ec, hf)][:, off : off + 512],
                            start=(ec == 0),
                            stop=(ec == NEC - 1),
                        )
                    nc.vector.tensor_scalar_add(
                        dst[:, rc * 512 : (rc + 1) * 512], pt[:], bsb[:, 0:1]
                    )

            def emit_v_proj(b, part):
                # part 0: DMAs + kvt 0..7 ; part 1: kvt 8..15
                if part == 0:
                    xt = {}
                    for ec in range(NEC):
                        for hf in range(2):
                            t = xin.tile([128, S // 2], BF16, tag="xin", name=f"xv{b}e{ec}h{hf}")
                            nc.sync.dma_start(
                                out=t[:],
                                in_=vT_d[
                                    ec * 128 : (ec + 1) * 128,
                                    b * S + hf * (S // 2) : b * S + (hf + 1) * (S // 2),
                                ],
                            )
                            xt[(ec, hf)] = t
                    v_xt[b] = xt
                xt = v_xt[b]
                for kvt in range(part * NKV // 2, (part + 1) * NKV // 2):
                    hf = kvt // (NKV // 2)
                    off = (kvt % (NKV // 2)) * 128
                    pv = ps.tile([128, 128], F32, tag="misc", bufs=2, name="pv")
                    for ec in range(NEC):
                        nc.tensor.matmul(
                            pv[:],
                            xt[(ec, hf)][:, off : off + 128],
                            wv_sb[:, ec * DC : (ec + 1) * DC],
                            start=(ec == 0),
                            stop=(ec == NEC - 1),
                        )
                    for h in range(2):
                        c0 = (kvt * 2 + h) * 65
                        nc.vector.tensor_copy(
                            v_sb[b][:, c0 : c0 + 64], pv[:, h * 64 : (h + 1) * 64]
                        )
                if part == 1:
                    del v_xt[b]

            def emit_proj_part(b, part):
                if part == 0:
                    emit_qk_proj(b, "q")
                elif part == 1:
                    emit_qk_proj(b, "k")
                else:
                    emit_v_proj(b, part - 2)

            def emit_attn_chunk(b, h, qc):
                d0 = h * 64
                qcol = qc * 512
                qTb, kTb, vb, ab = qT_sb[b], kT_sb[b], v_sb[b], attn_sb[b]
                et = []
                for kp in range(NKV2):
                    sps = ps.tile([128, 1024], F32, tag="sps", bufs=2, name="sps")
                    for j in range(2):
                        kvt = kp * 2 + j
                        nc.tensor.matmul(
                            sps[:, j * 512 : (j + 1) * 512],
                            kTb[d0 : d0 + 64, kvt * 128 : (kvt + 1) * 128],
                            qTb[d0 : d0 + 64, qcol : qcol + 512],
                            start=True,
                            stop=True,
                        )
                    e_t = expp.tile([128, 1024], BF16, tag="expp", name="et")
                    nc.scalar.activation(e_t[:], sps[:], AF.Exp, scale=0.125)
                    et.append(e_t)
                aps = ps.tile([65, 512], F32, tag="aps", bufs=2, name="aps")
                for kvt in range(NKV):
                    c0 = (kvt * 2 + h) * 65
                    nc.tensor.matmul(
                        aps[:],
                        vb[:, c0 : c0 + 65],
                        et[kvt // 2][:, (kvt % 2) * 512 : (kvt % 2 + 1) * 512],
                        start=(kvt == 0),
                        stop=(kvt == NKV - 1),
                    )
                rec = evp.tile([1, 512], F32, tag="rec", name="rec")
                with nc.allow_low_precision(reason="f32r denom broadcast"):
                    nc.vector.reciprocal(rec[:], aps[64:65, :])
                bps = ps.tile([64, 512], F32, tag="misc", bufs=2, name="bps")
                nc.tensor.matmul(bps[:], ones_sb[:, 0:64], rec[:], start=True, stop=True)
                bcs = evp.tile([64, 512], F32, tag="bc", name="bcs")
                nc.vector.tensor_copy(bcs[:], bps[:])
                nc.vector.tensor_mul(
                    ab[d0 : d0 + 64, qcol : qcol + 512], aps[0:64, :], bcs[:]
                )

            def emit_outproj(b, qc):
                ab = attn_sb[b]
                for qt in range(qc * 4, (qc + 1) * 4):
                    qcol = qt * 128
                    ot = otp.tile([128, EMBED], F32, tag="ot", name="ot")
                    for en in range(2):
                        po = ps.tile([128, 512], F32, tag="misc", bufs=2, name="po")
                        nc.tensor.matmul(
                            po[:],
                            ab[:, qcol : qcol + 128],
                            wo_sb[:, en * 512 : (en + 1) * 512],
                            start=True,
                            stop=True,
                        )
                        nc.vector.tensor_copy(ot[:, en * 512 : (en + 1) * 512], po[:])
                    nc.sync.dma_start(
                        out=out_d[b * S + qcol : b * S + qcol + 128, :], in_=ot[:]
                    )

            v_xt = {}
            # prologue: batch 0 projections
            alloc_batch(0)
            for part in range(4):
                emit_proj_part(0, part)
            # steady state: attention/outproj of b interleaved with proj of b+1
            for b in range(B):
                if b + 1 < B:
                    alloc_batch(b + 1)
                for qc in range(NQC):
                    emit_attn_chunk(b, 0, qc)
                    if b + 1 < B:
                        p0 = qc * 4 // NQC
                        p1 = (qc + 1) * 4 // NQC
                        for part in range(p0, p1):
                            emit_proj_part(b + 1, part)
                    emit_attn_chunk(b, 1, qc)
                    emit_outproj(b, qc)

    nc.finalize()
    return nc


_NC_CACHE = {}


def get_nc(B=4, S=2048):
    key = (B, S)
    if key not in _NC_CACHE:
        _NC_CACHE[key] = build_nc(B, S)
    return _NC_CACHE[key]


def make_in_maps(value, key, query, Wv, bv, Wk, bk, Wq, bq, Wo, bo, B, S):
    ROWS = B * S
    bf = ml_dtypes.bfloat16
    qTh = np.ascontiguousarray(query.reshape(ROWS, EMBED).astype(bf).T)
    kTh = np.ascontiguousarray(key.reshape(ROWS, EMBED).astype(bf).T)
    vTh = np.ascontiguousarray(value.reshape(ROWS, EMBED).astype(bf).T)
    in_maps = []
    for c in range(N_CORES):
        cs = slice(c * DC, (c + 1) * DC)
        in_maps.append(
            {
                "qT": qTh,
                "kT": kTh,
                "vT": vTh,
                "wq": np.ascontiguousarray(
                    Wq[:, cs].astype(bf).reshape(NEC, 128, DC)
                ),
                "wk": np.ascontiguousarray(
                    Wk[:, cs].astype(bf).reshape(NEC, 128, DC)
                ),
                "wv": np.ascontiguousarray(
                    Wv[:, cs].astype(bf).reshape(NEC, 128, DC)
                ),
                "bq": np.ascontiguousarray(bq[cs].reshape(DC, 1).astype(np.float32)),
                "bk": np.ascontiguousarray(bk[cs].reshape(DC, 1).astype(np.float32)),
                "wo": np.ascontiguousarray(Wo[cs, :].astype(bf)),
            }
        )
    return in_maps


def finish(results, Wv, bv, Wo, bo, B, S):
    acc = results[0]["out"].astype(np.float32).copy()
    for c in range(1, N_CORES):
        acc += results[c]["out"]
    acc += (bv.astype(np.float32) @ Wo.astype(np.float32) + bo.astype(np.float32))[
        None, :
    ]
    return acc.reshape(B, S, EMBED)


def kernel(value, key, query, Wv, bv, Wk, bk, Wq, bq, Wo, bo):
    B, S, _ = query.shape
    nc = get_nc(B, S)
    in_maps = make_in_maps(value, key, query, Wv, bv, Wk, bk, Wq, bq, Wo, bo, B, S)
    res = run_bass_kernel_spmd(nc, in_maps, list(range(N_CORES)))
    return finish(res.results, Wv, bv, Wo, bo, B, S)


# revision 15
# speedup vs baseline: 6.3309x; 6.3309x over previous
"""MultiHeadAttention TRN2 kernel — 8-way head-parallel (2 heads/core).

Sharding: core c owns heads 2c,2c+1 == columns c*128:(c+1)*128 of Wq/Wk/Wv
and rows c*128:(c+1)*128 of Wo. Each core computes its heads' projections,
attention, and a partial output projection; the host sums the 8 partials.

Device math (per core), all matmuls bf16 with f32 PSUM accumulation:
  qT/kT = (Wq_c^T x^T + bq_c)          feature-major [128, ROWS]
  v     = x^T^T Wv_c                   position-major [ROWS, 128]  (bv dropped:
                                        softmax rows sum to 1 => bv enters as the
                                        constant row bv@Wo, added on host)
  scoresT[kv, q] = kT^T qT             per (b, h), exp via ACT with scale=1/8
  attn_out[d, q] = sum_kv v_aug[kv, d] expT[kv, q]   (v_aug has a ones column
                                        => row 64 accumulates the softmax denom)
  attn = attn_out / denom              (DVE recip + PE broadcast + DVE mul)
  out_partial[q, e] = attn_cat^T Wo_c  f32 out
"""

import sys

sys.path.insert(0, "/opt/trn_rl_repo")

import numpy as np
import ml_dtypes

import concourse.bass as bass
from concourse import bacc
import concourse.mybir as mybir
from concourse.tile import TileContext
from concourse.bass_utils import run_bass_kernel_spmd

BF16 = mybir.dt.bfloat16
F32 = mybir.dt.float32
F32R = mybir.dt.float32r
AF = mybir.ActivationFunctionType

EMBED = 1024
HEADS = 16
HEAD_DIM = 64
N_CORES = 8
DC = 128  # feature columns per core (2 heads * 64)
NEC = 8  # contraction chunks of 128 over EMBED


def build_nc(B=4, S=2048):
    ROWS = B * S
    NRC = ROWS // 512  # row chunks for projections
    NQC = S // 512  # q chunks per (b, h)
    NKV = S // 128  # kv tiles per batch
    nc = bacc.Bacc("TRN2", target_bir_lowering=False)

    qT_d = nc.declare_dram_parameter("qT", [EMBED, ROWS], BF16, isOutput=False)
    kT_d = nc.declare_dram_parameter("kT", [EMBED, ROWS], BF16, isOutput=False)
    vT_d = nc.declare_dram_parameter("vT", [EMBED, ROWS], BF16, isOutput=False)
    wq_d = nc.declare_dram_parameter("wq", [128, NEC * DC], BF16, isOutput=False)
    wk_d = nc.declare_dram_parameter("wk", [128, NEC * DC], BF16, isOutput=False)
    wv_d = nc.declare_dram_parameter("wv", [128, NEC * DC], BF16, isOutput=False)
    bq_d = nc.declare_dram_parameter("bq", [DC, 1], F32, isOutput=False)
    bk_d = nc.declare_dram_parameter("bk", [DC, 1], F32, isOutput=False)
    wo_d = nc.declare_dram_parameter("wo", [DC, EMBED], BF16, isOutput=False)
    ones_d = nc.declare_dram_parameter("ones", [1, 64], F32R, isOutput=False)
    out_d = nc.declare_dram_parameter("out", [ROWS, EMBED], F32, isOutput=True)

    with TileContext(nc) as tc:
        with (
            tc.tile_pool(name="const", bufs=1) as cpool,
            tc.tile_pool(name="big", bufs=2) as big,
            tc.tile_pool(name="xin", bufs=24) as xin,
            tc.tile_pool(name="expp", bufs=16) as expp,
            tc.tile_pool(name="ev", bufs=4) as evp,
            tc.tile_pool(name="ot", bufs=4) as otp,
            tc.tile_pool(name="ps", bufs=1, space="PSUM") as ps,
        ):
            # --- weights / constants ---
            wq_sb = cpool.tile([128, NEC * DC], BF16, tag="wq")
            wk_sb = cpool.tile([128, NEC * DC], BF16, tag="wk")
            wv_sb = cpool.tile([128, NEC * DC], BF16, tag="wv")
            nc.sync.dma_start(out=wq_sb[:], in_=wq_d[:])
            nc.sync.dma_start(out=wk_sb[:], in_=wk_d[:])
            nc.sync.dma_start(out=wv_sb[:], in_=wv_d[:])
            wo_sb = cpool.tile([128, EMBED], BF16, tag="wo")
            nc.sync.dma_start(out=wo_sb[:], in_=wo_d[:])
            bq_sb = cpool.tile([128, 1], F32, tag="bq")
            bk_sb = cpool.tile([128, 1], F32, tag="bk")
            nc.sync.dma_start(out=bq_sb[:], in_=bq_d[:])
            nc.sync.dma_start(out=bk_sb[:], in_=bk_d[:])
            ones_sb = cpool.tile([1, 64], F32R, tag="ones")
            nc.sync.dma_start(out=ones_sb[:], in_=ones_d[:])

            NKV2 = NKV // 2  # paired kv tiles (exp processes two at once)

            qT_sb, kT_sb, v_sb, attn_sb = {}, {}, {}, {}

            def alloc_batch(b):
                qT_sb[b] = big.tile([128, S], BF16, tag="qTs", name=f"qTs{b}")
                kT_sb[b] = big.tile([128, S], BF16, tag="kTs", name=f"kTs{b}")
                v_sb[b] = big.tile([128, NKV * 2 * 65], BF16, tag="vs", name=f"vs{b}")
                attn_sb[b] = big.tile([128, S], BF16, tag="attns", name=f"attns{b}")
                nc.vector.memset(v_sb[b][:], 1.0)  # ones col (idx 64) per 65-block

            def emit_qk_proj(b, which):
                src_d, wsb, bsb = (
                    (qT_d, wq_sb, bq_sb) if which == "q" else (kT_d, wk_sb, bk_sb)
                )
                dst = (qT_sb if which == "q" else kT_sb)[b]
                tiles = {}
                for hf in range(2):
                    for ec in range(NEC):
                        t = xin.tile([128, S // 2], BF16, tag="xin", name=f"x{which}{b}e{ec}h{hf}")
                        nc.sync.dma_start(
                            out=t[:],
                            in_=src_d[
                                ec * 128 : (ec + 1) * 128,
                                b * S + hf * (S // 2) : b * S + (hf + 1) * (S // 2),
                            ],
                        )
                        tiles[(ec, hf)] = t
                rcph = max(1, S // 2 // 512)
                for rc in range(S // 512):
                    hf, off = rc // rcph, (rc % rcph) * 512
                    pt = ps.tile([128, 512], F32, tag="misc", bufs=2, name="pt")
                    for ec in range(NEC):
                        nc.tensor.matmul(
                            pt[:],
                            wsb[:, ec * DC : (ec + 1) * DC],
                            tiles[(ec, hf)][:, off : off + 512],
                            start=(ec == 0),
                            stop=(ec == NEC - 1),
                        )
                    nc.vector.tensor_scalar_add(
                        dst[:, rc * 512 : (rc + 1) * 512], pt[:], bsb[:, 0:1]
                    )

            def emit_v_proj(b, part):
                # part 0: DMAs + kvt 0..7 ; part 1: kvt 8..15
                if part == 0:
                    xt = {}
                    for ec in range(NEC):
                        for hf in range(2):
                            t = xin.tile([128, S // 2], BF16, tag="xin", name=f"xv{b}e{ec}h{hf}")
                            nc.sync.dma_start(
                                out=t[:],
                                in_=vT_d[
                                    ec * 128 : (ec + 1) * 128,
                                    b * S + hf * (S // 2) : b * S + (hf + 1) * (S // 2),
                                ],
                            )
                            xt[(ec, hf)] = t
                    v_xt[b] = xt
                xt = v_xt[b]
                for kvt in range(part * NKV // 2, (part + 1) * NKV // 2):
                    hf = kvt // (NKV // 2)
                    off = (kvt % (NKV // 2)) * 128
                    pv = ps.tile([128, 128], F32, tag="misc", bufs=2, name="pv")
                    for ec in range(NEC):
                        nc.tensor.matmul(
                            pv[:],
                            xt[(ec, hf)][:, off : off + 128],
                            wv_sb[:, ec * DC : (ec + 1) * DC],
                            start=(ec == 0),
                            stop=(ec == NEC - 1),
                        )
                    for h in range(2):
                        c0 = (kvt * 2 + h) * 65
                        nc.vector.tensor_copy(
                            v_sb[b][:, c0 : c0 + 64], pv[:, h * 64 : (h + 1) * 64]
                        )
                if part == 1:
                    del v_xt[b]

            def emit_proj_part(b, part):
                if part == 0:
                    emit_qk_proj(b, "q")
                elif part == 1:
                    emit_qk_proj(b, "k")
                else:
                    emit_v_proj(b, part - 2)

            def emit_attn_chunk(b, h, qc):
                d0 = h * 64
                qcol = qc * 512
                qTb, kTb, vb, ab = qT_sb[b], kT_sb[b], v_sb[b], attn_sb[b]
                et = []
                for kp in range(NKV2):
                    sps = ps.tile([128, 1024], F32, tag="sps", bufs=2, name="sps")
                    for j in range(2):
                        kvt = kp * 2 + j
                        nc.tensor.matmul(
                            sps[:, j * 512 : (j + 1) * 512],
                            kTb[d0 : d0 + 64, kvt * 128 : (kvt + 1) * 128],
                            qTb[d0 : d0 + 64, qcol : qcol + 512],
                            start=True,
                            stop=True,
                        )
                    e_t = expp.tile([128, 1024], BF16, tag="expp", name="et")
                    nc.scalar.activation(e_t[:], sps[:], AF.Exp, scale=0.125)
                    et.append(e_t)
                aps = ps.tile([65, 512], F32, tag="aps", bufs=2, name="aps")
                for kvt in range(NKV):
                    c0 = (kvt * 2 + h) * 65
                    nc.tensor.matmul(
                        aps[:],
                        vb[:, c0 : c0 + 65],
                        et[kvt // 2][:, (kvt % 2) * 512 : (kvt % 2 + 1) * 512],
                        start=(kvt == 0),
                        stop=(kvt == NKV - 1),
                    )
                rec = evp.tile([1, 512], F32R, tag="rec", name="rec")
                with nc.allow_low_precision(reason="f32r denom broadcast"):
                    nc.vector.reciprocal(rec[:], aps[64:65, :])
                bps = ps.tile([64, 512], F32, tag="misc", bufs=2, name="bps")
                nc.tensor.matmul(bps[:], ones_sb[:, 0:64], rec[:], start=True, stop=True)
                bcs = evp.tile([64, 512], F32, tag="bc", name="bcs")
                nc.vector.tensor_copy(bcs[:], bps[:])
                nc.vector.tensor_mul(
                    ab[d0 : d0 + 64, qcol : qcol + 512], aps[0:64, :], bcs[:]
                )

            def emit_outproj(b, qc):
                ab = attn_sb[b]
                for qt in range(qc * 4, (qc + 1) * 4):
                    qcol = qt * 128
                    ot = otp.tile([128, EMBED], F32, tag="ot", name="ot")
                    for en in range(2):
                        po = ps.tile([128, 512], F32, tag="misc", bufs=2, name="po")
                        nc.tensor.matmul(
                            po[:],
                            ab[:, qcol : qcol + 128],
                            wo_sb[:, en * 512 : (en + 1) * 512],
                            start=True,
                            stop=True,
                        )
                        nc.vector.tensor_copy(ot[:, en * 512 : (en + 1) * 512], po[:])
                    nc.sync.dma_start(
                        out=out_d[b * S + qcol : b * S + qcol + 128, :], in_=ot[:]
                    )

            v_xt = {}
            # prologue: batch 0 projections
            alloc_batch(0)
            for part in range(4):
                emit_proj_part(0, part)
            # steady state: attention/outproj of b interleaved with proj of b+1
            for b in range(B):
                if b + 1 < B:
                    alloc_batch(b + 1)
                for qc in range(NQC):
                    emit_attn_chunk(b, 0, qc)
                    if b + 1 < B:
                        p0 = qc * 4 // NQC
                        p1 = (qc + 1) * 4 // NQC
                        for part in range(p0, p1):
                            emit_proj_part(b + 1, part)
                    emit_attn_chunk(b, 1, qc)
                    emit_outproj(b, qc)

    nc.finalize()
    return nc


_NC_CACHE = {}


def get_nc(B=4, S=2048):
    key = (B, S)
    if key not in _NC_CACHE:
        _NC_CACHE[key] = build_nc(B, S)
    return _NC_CACHE[key]


def make_in_maps(value, key, query, Wv, bv, Wk, bk, Wq, bq, Wo, bo, B, S):
    ROWS = B * S
    bf = ml_dtypes.bfloat16
    qTh = np.ascontiguousarray(query.reshape(ROWS, EMBED).astype(bf).T)
    kTh = np.ascontiguousarray(key.reshape(ROWS, EMBED).astype(bf).T)
    vTh = np.ascontiguousarray(value.reshape(ROWS, EMBED).astype(bf).T)
    in_maps = []
    for c in range(N_CORES):
        cs = slice(c * DC, (c + 1) * DC)
        in_maps.append(
            {
                "qT": qTh,
                "kT": kTh,
                "vT": vTh,
                "wq": np.ascontiguousarray(
                    Wq[:, cs].astype(bf).reshape(NEC, 128, DC).transpose(1, 0, 2).reshape(128, NEC * DC)
                ),
                "wk": np.ascontiguousarray(
                    Wk[:, cs].astype(bf).reshape(NEC, 128, DC).transpose(1, 0, 2).reshape(128, NEC * DC)
                ),
                "wv": np.ascontiguousarray(
                    Wv[:, cs].astype(bf).reshape(NEC, 128, DC).transpose(1, 0, 2).reshape(128, NEC * DC)
                ),
                "bq": np.ascontiguousarray(bq[cs].reshape(DC, 1).astype(np.float32)),
                "bk": np.ascontiguousarray(bk[cs].reshape(DC, 1).astype(np.float32)),
                "wo": np.ascontiguousarray(Wo[cs, :].astype(bf)),
                "ones": np.ones((1, 64), np.float32),
            }
        )
    return in_maps


def finish(results, Wv, bv, Wo, bo, B, S):
    acc = results[0]["out"].astype(np.float32).copy()
    for c in range(1, N_CORES):
        acc += results[c]["out"]
    acc += (bv.astype(np.float32) @ Wo.astype(np.float32) + bo.astype(np.float32))[
        None, :
    ]
    return acc.reshape(B, S, EMBED)


def kernel(value, key, query, Wv, bv, Wk, bk, Wq, bq, Wo, bo):
    B, S, _ = query.shape
    nc = get_nc(B, S)
    in_maps = make_in_maps(value, key, query, Wv, bv, Wk, bk, Wq, bq, Wo, bo, B, S)
    res = run_bass_kernel_spmd(nc, in_maps, list(range(N_CORES)))
    return finish(res.results, Wv, bv, Wo, bo, B, S)
